# revision 9
# baseline (speedup 1.0000x reference)
"""GCNII forward on 8 TRN2 NeuronCores (self-contained).

Strategy (1D row partitioning):
- nodes sharded 2500/core (padded 2560); edges assigned to the core owning dst.
- per layer: ht = dinv*h exchanged as fp8e4m3 via three sectioned AllGathers
  (tiles 0-7 / 8-13 / 14-19) into a shared DRAM table [20480,1024] so most of
  the exchange overlaps tile compute; each core dma_gathers its edges' source
  rows (768 rows per SWDGE instruction, 3 per dst tile), scatter-adds them
  into per-dst-tile PSUM via one-hot fp8 DoubleRow matmuls (256 edges per
  accumulation step), then z = 0.9*dinv*agg + 0.1*h0 (h0 SBUF-resident, f16)
  and the layer GEMM z @ ((1-b)I + b*W) in f16 with the identity residual
  folded into the weights on the host. z transposed via f16 PE transposes.
  The tile loop is software-pipelined: transposes/GEMM/exchange of tile t-1
  are emitted after the scatter matmuls of tile t so the PE never waits on
  the DVE z computation.
- self-loops are real edges; gcn_norm folded into per-node dinv scaling.
"""
import math
import numpy as np

import concourse.bass as bass
import concourse.mybir as mybir
import concourse.tile as tile
from concourse import bacc
from concourse.bass_utils import run_bass_kernel_spmd
from concourse.masks import make_identity

# problem constants (hardcoded per contract)
N, E = 20000, 320000
F_IN, H, C, L = 512, 1024, 64, 8
ALPHA, THETA = 0.1, 0.5
NCORES = 8
SH = N // NCORES          # 2500 real rows per core
SHP = 2560                # padded rows per core (20*128)
V = NCORES * SHP          # padded table rows
P = 128
NT = SHP // P             # 20 dst tiles per core
KF = F_IN // P            # 4 k-tiles for W1
KH = H // P               # 8 k-tiles for H
GK = 6                    # chunks gathered per dma_gather (768 rows)

# AllGather sections (per-core row ranges): tiles 0-7, 8-13, 14-19
SEC_BASE = (0, 1024, 1792)
SEC_SIZE = (1024, 768, 768)
SEC_GBASE = (0, 8 * 1024, 8 * 1024 + 8 * 768)
SEC_LAST_TILE = (7, 13, 19)

f32 = mybir.dt.float32
f16 = mybir.dt.float16
f8 = mybir.dt.float8e4
i16 = mybir.dt.int16

_cache = {}


def _f8(a):
    import ml_dtypes
    return np.asarray(a, np.float32).astype(ml_dtypes.float8_e4m3fn).view(
        np.uint8)


def _preprocess(x, edge_index, W1, b1, Wg, W2, b2):
    src = np.asarray(edge_index[0], dtype=np.int64)
    dst = np.asarray(edge_index[1], dtype=np.int64)
    loops = np.arange(N, dtype=np.int64)
    src = np.concatenate([src, loops])
    dst = np.concatenate([dst, loops])
    deg = np.bincount(dst, minlength=N).astype(np.float32)
    dinv = 1.0 / np.sqrt(np.maximum(deg, 1.0))

    core = dst // SH
    d_loc = dst - core * SH
    tl = d_loc // P
    slot = d_loc % P
    gid = core * NT + tl
    order = np.argsort(gid, kind="stable")
    gid_s = gid[order]
    src_s = src[order]
    slot_s = slot[order]
    counts = np.bincount(gid_s, minlength=NCORES * NT)
    starts = np.concatenate([[0], np.cumsum(counts)[:-1]])
    j = np.arange(len(gid_s)) - starts[gid_s]
    nchunk = int(math.ceil(counts.max() / P))
    nchunk = GK * int(math.ceil(nchunk / GK))  # pad to gather batch multiple
    c_idx = j // P
    p_idx = j % P
    # sectioned table row for global node owned by core c at local i
    s_core = src_s // SH
    s_loc = src_s - s_core * SH
    s_sec = np.select([s_loc < 1024, s_loc < 1792], [0, 1], 2)
    sb = np.asarray(SEC_BASE)[s_sec]
    ssz = np.asarray(SEC_SIZE)[s_sec]
    sgb = np.asarray(SEC_GBASE)[s_sec]
    s_tab = (sgb + s_core * ssz + (s_loc - sb)).astype(np.int32)

    offs = np.zeros((NCORES, P, NT * nchunk), dtype=np.int32)
    S = np.zeros((NCORES, NT, P, nchunk, P), dtype=np.float32)
    core_s = gid_s // NT
    tl_s = gid_s % NT
    offs[core_s, p_idx, tl_s * nchunk + c_idx] = s_tab
    S[core_s, tl_s, p_idx, c_idx, slot_s] = 1.0
    S = S.reshape(NCORES, NT, P, nchunk * P)
    # dma_gather idxs: per tile, flat order i = chunk*128 + slot, wrapped
    # into 16 partitions ([i%16, i//16]) and replicated to 128.
    ncols = nchunk * P // 16
    idxs = np.zeros((NCORES, 16, NT * ncols), dtype=np.int16)
    ii = np.arange(nchunk * P)
    for c in range(NCORES):
        flat_all = offs[c].reshape(P, NT, nchunk).transpose(1, 2, 0)
        for t in range(NT):
            f = flat_all[t].reshape(-1)
            w = np.zeros((16, ncols), np.int16)
            w[ii % 16, ii // 16] = f.astype(np.int16)
            idxs[c, :, t * ncols:(t + 1) * ncols] = w
    idxs = np.tile(idxs, (1, 8, 1))

    dinv_pad = np.zeros(NCORES * SHP, dtype=np.float32)
    idx = np.arange(N)
    dinv_pad[(idx // SH) * SHP + (idx % SH)] = dinv
    dinvc = dinv_pad.reshape(NCORES, NT, P).transpose(0, 2, 1).copy()
    dinv09c = (0.9 * dinvc).astype(np.float32)

    x = np.asarray(x, dtype=np.float32)
    xT = np.zeros((NCORES, F_IN, SHP), dtype=np.float16)
    for c in range(NCORES):
        xT[c, :, :SH] = x[c * SH:(c + 1) * SH].T.astype(np.float16)

    betas = np.log(THETA / np.arange(1.0, L + 1.0, dtype=np.float64) + 1.0)
    Wg = np.asarray(Wg, dtype=np.float64)
    eye = np.eye(H, dtype=np.float64)
    Wt = np.stack([(1.0 - betas[l]) * eye + betas[l] * Wg[l] for l in range(L)])
    Wt = Wt.astype(np.float16)

    b1b = np.broadcast_to(np.asarray(b1, np.float32), (P, H)).copy()
    b2b = np.broadcast_to(np.asarray(b2, np.float32), (P, C)).copy()

    in_maps = []
    for c in range(NCORES):
        in_maps.append({
            "xT": xT[c],
            "W1": np.asarray(W1, np.float16),
            "Wt": Wt,
            "W2": np.asarray(W2, np.float16),
            "b1b": b1b,
            "b2b": b2b,
            "dinvc": dinvc[c],
            "dinv09c": dinv09c[c],
            "idxs": idxs[c],
            "Smat": _f8(S[c]),
        })
    return in_maps, nchunk


def _build(nchunk):
    npair = nchunk // 2
    nc = bacc.Bacc("TRN2", target_bir_lowering=False, debug=False,
                   num_devices=NCORES)
    t_xT = nc.dram_tensor("xT", [F_IN, SHP], f16, kind="ExternalInput")
    t_W1 = nc.dram_tensor("W1", [F_IN, H], f16, kind="ExternalInput")
    t_Wt = nc.dram_tensor("Wt", [L, H, H], f16, kind="ExternalInput")
    t_W2 = nc.dram_tensor("W2", [H, C], f16, kind="ExternalInput")
    t_b1 = nc.dram_tensor("b1b", [P, H], f32, kind="ExternalInput")
    t_b2 = nc.dram_tensor("b2b", [P, C], f32, kind="ExternalInput")
    t_dinv = nc.dram_tensor("dinvc", [P, NT], f32, kind="ExternalInput")
    t_dinv09 = nc.dram_tensor("dinv09c", [P, NT], f32, kind="ExternalInput")
    t_idx = nc.dram_tensor("idxs", [P, NT * nchunk * P // 16], i16,
                           kind="ExternalInput")
    t_S = nc.dram_tensor("Smat", [NT, P, nchunk * P], f8, kind="ExternalInput")
    t_out = nc.dram_tensor("out", [SHP, C], f32, kind="ExternalOutput")

    exch_in = nc.dram_tensor("exch", [SHP, H], f8)
    tables = [nc.dram_tensor(f"tbl{i}", [V, H], f8, addr_space="Shared")
              for i in range(2)]

    with tile.TileContext(nc) as tc:
        with (
            tc.tile_pool(name="const", bufs=1) as cp,
            tc.tile_pool(name="wpool", bufs=2) as wp,
            tc.tile_pool(name="gpool", bufs=3) as gp,
            tc.tile_pool(name="spool", bufs=3) as sp,
            tc.tile_pool(name="zpool", bufs=2) as zp,
            tc.tile_pool(name="ps_agg", bufs=2, space="PSUM") as pa,
            tc.tile_pool(name="ps_gemm", bufs=1, space="PSUM") as pg,
            tc.tile_pool(name="ps_tr", bufs=2, space="PSUM") as pt,
        ):
            ident = cp.tile([P, P], f16, tag="ident")
            make_identity(nc, ident[:])
            idx_sb = cp.tile([P, NT * nchunk * P // 16], i16, tag="idx")
            nc.sync.dma_start(out=idx_sb[:], in_=t_idx[:])
            dinv_sb = cp.tile([P, NT], f32, tag="dinv")
            nc.sync.dma_start(out=dinv_sb[:], in_=t_dinv[:])
            dinv09_sb = cp.tile([P, NT], f32, tag="dinv09")
            nc.sync.dma_start(out=dinv09_sb[:], in_=t_dinv09[:])
            b1_sb = cp.tile([P, H], f32, tag="b1")
            nc.sync.dma_start(out=b1_sb[:], in_=t_b1[:])
            b2_sb = cp.tile([P, C], f32, tag="b2")
            nc.sync.dma_start(out=b2_sb[:], in_=t_b2[:])
            W2_sb = cp.tile([P, KH * C], f16, tag="W2")
            for k in range(KH):
                nc.scalar.dma_start(out=W2_sb[:, k * C:(k + 1) * C],
                                    in_=t_W2[k * P:(k + 1) * P, :])
            h0_sb = cp.tile([P, NT * H], f16, tag="h0")

            def emit_ag(sec, dst_tbl):
                base, size, gb = SEC_BASE[sec], SEC_SIZE[sec], SEC_GBASE[sec]
                nc.gpsimd.collective_compute(
                    "AllGather", mybir.AluOpType.bypass,
                    replica_groups=[list(range(NCORES))],
                    ins=[exch_in[base:base + size, :]],
                    outs=[dst_tbl[gb:gb + NCORES * size, :]])

            def emit_exch(t, ps, dst_tbl):
                ex_t = zp.tile([P, H], f8, tag="ex")
                nc.scalar.activation(out=ex_t[:], in_=ps[:],
                                     func=mybir.ActivationFunctionType.Relu,
                                     scale=dinv_sb[:, t:t + 1])
                nc.sync.dma_start(out=exch_in[t * P:(t + 1) * P, :],
                                  in_=ex_t[:])
                for sec in range(3):
                    if t == SEC_LAST_TILE[sec]:
                        emit_ag(sec, dst_tbl)

            # ---- phase 0: h0 = relu(x@W1 + b1); table0 = f8(dinv*h0)
            xT_sb = cp.tile([P, KF * SHP], f16, tag="xT")
            for k in range(KF):
                nc.sync.dma_start(out=xT_sb[:, k * SHP:(k + 1) * SHP],
                                  in_=t_xT[k * P:(k + 1) * P, :])
            W1_sb = cp.tile([P, KF * H], f16, tag="W1")
            for k in range(KF):
                nc.scalar.dma_start(out=W1_sb[:, k * H:(k + 1) * H],
                                    in_=t_W1[k * P:(k + 1) * P, :])
            for t in range(NT):
                ps = pg.tile([P, H], f32, space="PSUM", tag="gemm")
                for k in range(KF):
                    for nh in range(2):
                        nc.tensor.matmul(
                            out=ps[:, nh * 512:(nh + 1) * 512],
                            lhsT=xT_sb[:, k * SHP + t * P: k * SHP + (t + 1) * P],
                            rhs=W1_sb[:, k * H + nh * 512: k * H + (nh + 1) * 512],
                            start=(k == 0), stop=(k == KF - 1))
                nc.vector.tensor_add(out=ps[:], in0=ps[:], in1=b1_sb[:])
                nc.scalar.activation(out=h0_sb[:, t * H:(t + 1) * H], in_=ps[:],
                                     func=mybir.ActivationFunctionType.Relu,
                                     scale=0.1)
                emit_exch(t, ps, tables[0])

            # ---- layers (software-pipelined tile loop)
            for l in range(L):
                tbl = tables[l % 2]
                W_sb = wp.tile([P, KH * H], f16, tag="W")
                for k in range(KH):
                    nc.scalar.dma_start(out=W_sb[:, k * H:(k + 1) * H],
                                        in_=t_Wt[l, k * P:(k + 1) * P, :])

                def front(t, tbl=tbl):
                    agg = pa.tile([P, H], f32, space="PSUM", tag="agg")
                    S_sb = sp.tile([P, nchunk * P], f8, tag="S")
                    nc.scalar.dma_start(out=S_sb[:], in_=t_S[t])
                    S3 = S_sb[:].rearrange("p (c d) -> p c d", c=nchunk)
                    g_sb = gp.tile([P, nchunk * H], f8, tag="g")
                    g3 = g_sb[:].rearrange("p (c h) -> p c h", c=nchunk)
                    tc0 = t * (nchunk * P // 16)
                    for gj in range(nchunk // GK):
                        nc.gpsimd.dma_gather(
                            g3[:, gj * GK:(gj + 1) * GK, :], tbl.ap(),
                            idx_sb[:, tc0 + gj * (GK * P // 16):
                                   tc0 + (gj + 1) * (GK * P // 16)],
                            GK * P, GK * P, H)
                    for pr in range(npair):
                        for nh in range(2):
                            nc.tensor.matmul(
                                out=agg[:, nh * 512:(nh + 1) * 512],
                                lhsT=S3[:, 2 * pr:2 * pr + 2, :],
                                rhs=g3[:, 2 * pr:2 * pr + 2,
                                       nh * 512:(nh + 1) * 512],
                                perf_mode=mybir.MatmulPerfMode.DoubleRow,
                                start=(pr == 0), stop=(pr == npair - 1))
                    # z = 0.9*dinv*agg + 0.1*h0   (f16)
                    z0 = zp.tile([P, H], f16, tag="z0")
                    nc.vector.tensor_scalar(
                        out=z0[:], in0=agg[:], scalar1=dinv09_sb[:, t:t + 1],
                        scalar2=None, op0=mybir.AluOpType.mult)
                    z = zp.tile([P, H], f16, tag="z")
                    nc.vector.tensor_add(out=z[:], in0=z0[:],
                                         in1=h0_sb[:, t * H:(t + 1) * H])
                    return z

                def back(t, z, l=l):
                    zT = zp.tile([P, KH * P], f16, tag="zT")
                    trp = pt.tile([P, KH * P], f16, space="PSUM", tag="tr")
                    for k in range(KH):
                        nc.tensor.transpose(out=trp[:, k * P:(k + 1) * P],
                                            in_=z[:, k * P:(k + 1) * P],
                                            identity=ident[:])
                    nc.vector.tensor_copy(out=zT[:], in_=trp[:])
                    ps = pg.tile([P, H], f32, space="PSUM", tag="gemm")
                    for k in range(KH):
                        for nh in range(2):
                            nc.tensor.matmul(
                                out=ps[:, nh * 512:(nh + 1) * 512],
                                lhsT=zT[:, k * P:(k + 1) * P],
                                rhs=W_sb[:, k * H + nh * 512:
                                         k * H + (nh + 1) * 512],
                                start=(k == 0), stop=(k == KH - 1))
                    if l < L - 1:
                        emit_exch(t, ps, tables[(l + 1) % 2])
                    else:
                        # logits -> log_softmax -> out
                        h8 = zp.tile([P, H], f16, tag="z")
                        nc.scalar.activation(
                            out=h8[:], in_=ps[:],
                            func=mybir.ActivationFunctionType.Relu)
                        hT = zp.tile([P, KH * P], f16, tag="zT")
                        trp2 = pt.tile([P, KH * P], f16, space="PSUM",
                                       tag="tr")
                        for k in range(KH):
                            nc.tensor.transpose(
                                out=trp2[:, k * P:(k + 1) * P],
                                in_=h8[:, k * P:(k + 1) * P],
                                identity=ident[:])
                        nc.vector.tensor_copy(out=hT[:], in_=trp2[:])
                        psf = pg.tile([P, H], f32, space="PSUM", tag="gemm")
                        psl = psf[:, 0:C]
                        for k in range(KH):
                            nc.tensor.matmul(
                                out=psl,
                                lhsT=hT[:, k * P:(k + 1) * P],
                                rhs=W2_sb[:, k * C:(k + 1) * C],
                                start=(k == 0), stop=(k == KH - 1))
                        nc.vector.tensor_add(out=psl, in0=psl, in1=b2_sb[:])
                        mx = zp.tile([P, 1], f32, tag="mx")
                        nc.vector.tensor_reduce(out=mx[:], in_=psl,
                                                axis=mybir.AxisListType.X,
                                                op=mybir.AluOpType.max)
                        nmx = zp.tile([P, 1], f32, tag="nmx")
                        nc.vector.tensor_scalar(
                            out=nmx[:], in0=mx[:], scalar1=-1.0, scalar2=None,
                            op0=mybir.AluOpType.mult)
                        esb = zp.tile([P, C], f32, tag="esb")
                        se = zp.tile([P, 1], f32, tag="se")
                        nc.scalar.activation(
                            out=esb[:], in_=psl,
                            func=mybir.ActivationFunctionType.Exp,
                            bias=nmx[:], accum_out=se[:])
                        lse = zp.tile([P, 1], f32, tag="lse")
                        nc.scalar.activation(
                            out=lse[:], in_=se[:],
                            func=mybir.ActivationFunctionType.Ln)
                        o_t = zp.tile([P, C], f32, tag="ot")
                        nc.vector.tensor_scalar(
                            out=o_t[:], in0=psl, scalar1=mx[:],
                            scalar2=lse[:],
                            op0=mybir.AluOpType.subtract,
                            op1=mybir.AluOpType.subtract)
                        nc.sync.dma_start(out=t_out[t * P:(t + 1) * P, :],
                                          in_=o_t[:])

                prev = None
                for t in range(NT):
                    z = front(t)
                    if prev is not None:
                        back(prev[0], prev[1])
                    prev = (t, z)
                back(prev[0], prev[1])
    nc.compile()
    return nc


def kernel(**inputs):
    in_maps, nchunk = _preprocess(
        inputs["x"], inputs["edge_index"], inputs["W1"], inputs["b1"],
        inputs["Wg"], inputs["W2"], inputs["b2"])
    key = ("nc", nchunk)
    if key not in _cache:
        _cache[key] = _build(nchunk)
    nc = _cache[key]
    res = run_bass_kernel_spmd(nc, in_maps, list(range(NCORES)))
    out = np.concatenate(
        [res.results[c]["out"][:SH] for c in range(NCORES)], axis=0)
    return out.astype(np.float32)
